# revision 1
# baseline (speedup 1.0000x reference)
"""BiasedMHA Trainium2 kernel.

Full inputs -> shard batch over 8 NeuronCores -> Bass/Tile kernel -> gather.

Reference semantics (B=16, N=512, F=512, H=16, D=32):
  q = (x @ Wq.T + bq) * sqrt(D); k = x @ Wk.T + bk; v = x @ Wv.T + bv
  s[b,q,k,h] = sum_d q.k + bias[b,q,k,h];  s = -inf where mask[b,q,k]!=0
  p = softmax_k(s);  out = (p @ v reshaped) @ Wo.T + bo

Per-core design notes:
 - X^T via PE transpose; projections as W^T-stationary fp32 matmuls.
 - V kept in natural (n, f) layout, augmented with a ones column per head so
   the P@V matmul also emits the softmax denominator (M=33) for free.
 - Scores stay q-major for the (q,k,h)-contiguous bias tile add + int mask
   predication, then are PE-transposed to k-major so the exp (ScalarE) writes
   P^T straight to SBUF for the P@V matmul - no PSUM->SBUF copy for P.
 - softmax uses a fixed exp shift (exp(s - C)) instead of a row max: scores
   are bounded (std ~16) so exp stays in fp32 range and the shift cancels.
 - Normalization (1/rowsum) is folded into the attn^T PSUM->SBUF copies.
"""

import os
import numpy as np
from contextlib import ExitStack

import concourse.bass as bass
import concourse.mybir as mybir
import concourse.tile as tile
from concourse import bacc
from concourse.bass_utils import run_bass_kernel_spmd
from concourse.masks import make_identity

F32 = mybir.dt.float32
F32R = mybir.dt.float32r
I32 = mybir.dt.int32
ADD = mybir.AluOpType.add
MULT = mybir.AluOpType.mult
AF = mybir.ActivationFunctionType

B, N, F, H = 16, 512, 512, 16
D = F // H            # 32
NCORES = 8
BLOC = B // NCORES    # 2
P = 128
QT = N // P           # 4 q tiles
KC = N // P           # 4 k chunks
SQRT_D = float(np.sqrt(D))
C_EXP = 90.0          # fixed softmax shift; |scores| << C_EXP + 87 (fp32 safe)
NEG_HUGE = -1.0e30


def _emit(nc, tc, ctx, t, reps=1):
    consts = ctx.enter_context(tc.tile_pool(name="consts", bufs=1))
    wpool = ctx.enter_context(tc.tile_pool(name="weights", bufs=1))
    xpool = ctx.enter_context(tc.tile_pool(name="x", bufs=5))
    bpool = ctx.enter_context(tc.tile_pool(name="perbatch", bufs=1))
    biaspool = ctx.enter_context(tc.tile_pool(name="bias", bufs=2))
    maskpool = ctx.enter_context(tc.tile_pool(name="mask", bufs=2))
    mcpool = ctx.enter_context(tc.tile_pool(name="maskC", bufs=2))
    spool = ctx.enter_context(tc.tile_pool(name="sprime", bufs=4))
    ppool = ctx.enter_context(tc.tile_pool(name="pT", bufs=4))
    atsb = ctx.enter_context(tc.tile_pool(name="attnT", bufs=2))
    opool = ctx.enter_context(tc.tile_pool(name="o", bufs=2))
    rspool = ctx.enter_context(tc.tile_pool(name="rs", bufs=2))

    ps_sc = ctx.enter_context(tc.tile_pool(name="ps_sc", bufs=3, space="PSUM"))
    ps_at = ctx.enter_context(tc.tile_pool(name="ps_at", bufs=3, space="PSUM"))
    ps_mi = ctx.enter_context(tc.tile_pool(name="ps_mi", bufs=2, space="PSUM"))

    ident = consts.tile([P, P], F32)
    make_identity(nc, ident[:])
    neghuge = consts.tile([P, 1], F32)
    nc.vector.memset(neghuge[:], NEG_HUGE)
    ones_col = consts.tile([1, P], F32)
    nc.vector.memset(ones_col[:], 1.0)
    ones_r = consts.tile([1, P], F32R)
    nc.vector.tensor_copy(ones_r[:], ones_col[:])
    negc = consts.tile([P, 1], F32)
    nc.vector.memset(negc[:], -C_EXP)

    # per-partition bias vectors for Q/K projection epilogues
    bqs_sb = consts.tile([P, 4], F32)
    nc.sync.dma_start(bqs_sb[:], t["bqs"].rearrange("(a p) -> p a", p=P))
    bk_sb = consts.tile([P, 4], F32)
    nc.sync.dma_start(bk_sb[:], t["bk"].rearrange("(a p) -> p a", p=P))
    bv_row0 = consts.tile([1, F], F32)
    nc.sync.dma_start(bv_row0[:], t["bv"].rearrange("(a f) -> a f", a=1))
    bv_row = consts.tile([1, F], F32R)
    nc.vector.tensor_copy(bv_row[:], bv_row0[:])
    bo_row0 = consts.tile([1, F], F32)
    nc.sync.dma_start(bo_row0[:], t["bo"].rearrange("(a f) -> a f", a=1))
    bo_row = consts.tile([1, F], F32R)
    nc.vector.tensor_copy(bo_row[:], bo_row0[:])

    # prefetch the first batch's X tiles ahead of the (bulky) weight DMAs so
    # the PE transposes can start immediately
    x_prefetch = []
    for nb in range(4):
        xt_ = xpool.tile([P, F], F32, tag="x", name=f"xpre{nb}")
        nc.sync.dma_start(xt_[:], t["nfeat"][0, P * nb : P * (nb + 1), :])
        x_prefetch.append(xt_)
    bias_pre = biaspool.tile([P, N, H], F32, tag="bias", name="biaspre")
    nc.sync.dma_start(bias_pre[:], t["attn_bias"][0, 0:P, :, :])
    mask_pre = maskpool.tile([P, N], I32, tag="mask", name="maskpre")
    nc.sync.dma_start(mask_pre[:], t["attn_mask"][0, 0:P, :])

    w_sb = {}
    for name in ("wqT", "wkT", "wvT", "woT"):
        w_sb[name] = []
        for ki in range(4):
            wt = wpool.tile([P, F], F32, tag=f"{name}{ki}")
            nc.sync.dma_start(wt[:], t[name][P * ki : P * (ki + 1), :])
            w_sb[name].append(wt)

    for rep in range(reps):
      for b in range(BLOC):
        # ---- X load + transpose to (f_in, n)
        if rep == 0 and b == 0:
            x_tiles = x_prefetch
        else:
            x_tiles = []
            for nb in range(4):
                xt_ = xpool.tile([P, F], F32, tag="x")
                nc.sync.dma_start(xt_[:], t["nfeat"][b, P * nb : P * (nb + 1), :])
                x_tiles.append(xt_)
        xT_sb = bpool.tile([P, 4, N], F32, tag="xT")
        for fb in range(4):
            ps = ps_mi.tile([P, N], F32, tag="mi")
            for nb in range(4):
                nc.tensor.transpose(
                    ps[:, P * nb : P * (nb + 1)],
                    x_tiles[nb][:, P * fb : P * (fb + 1)],
                    ident[:],
                )
            nc.scalar.copy(xT_sb[:, fb, :], ps[:])

        # ---- Q/K projections -> (f_out, n), V -> natural (n, f) augmented
        qT_sb = bpool.tile([P, 4, N], F32, tag="qT", bufs=2)
        kT_sb = bpool.tile([P, 4, N], F32, tag="kT")
        for wname, dest, scale, bvec in (
            ("wqT", qT_sb, SQRT_D, bqs_sb),
            ("wkT", kT_sb, 1.0, bk_sb),
        ):
            for fo in range(4):
                ps = ps_mi.tile([P, N], F32, tag="mi")
                for ki in range(4):
                    nc.tensor.matmul(
                        ps[:],
                        w_sb[wname][ki][:, P * fo : P * (fo + 1)],
                        xT_sb[:, ki, :],
                        start=(ki == 0),
                        stop=(ki == 3),
                    )
                nc.scalar.activation(
                    dest[:, fo, :], ps[:], AF.Identity,
                    bias=bvec[:, fo : fo + 1], scale=scale,
                )

        v_aug = bpool.tile([P, 4, H, 2 * D], F32, tag="vaug")
        for nb in range(4):
            ps = ps_mi.tile([P, N], F32, tag="mi")
            for ki in range(4):
                nc.tensor.matmul(
                    ps[:],
                    xT_sb[:, ki, P * nb : P * (nb + 1)],
                    w_sb["wvT"][ki][:],
                    start=(ki == 0),
                    stop=False,
                )
            nc.tensor.matmul(ps[:], ones_r[:], bv_row[:], start=False, stop=True)
            nc.scalar.copy(
                v_aug[:, nb, :, 0:D], ps[:].rearrange("p (h d) -> p h d", h=H)
            )
            nc.vector.memset(v_aug[:, nb, :, D : 2 * D], 1.0)

        # ---- attention per q-tile
        for qt in range(QT):
            if rep == 0 and b == 0 and qt == 0:
                bias_t, mask_t = bias_pre, mask_pre
            else:
                bias_t = biaspool.tile([P, N, H], F32, tag="bias")
                nc.sync.dma_start(bias_t[:], t["attn_bias"][b, P * qt : P * (qt + 1), :, :])
                mask_t = maskpool.tile([P, N], I32, tag="mask")
                nc.sync.dma_start(mask_t[:], t["attn_mask"][b, P * qt : P * (qt + 1), :])
            maskf = mcpool.tile([P, N], F32, tag="maskf")
            nc.gpsimd.tensor_copy(maskf[:], mask_t[:])
            maskC = mcpool.tile([P, N, 1], F32, tag="maskC")
            nc.vector.tensor_scalar(
                maskC[:, :, 0], maskf[:], 0.0, NEG_HUGE,
                op0=mybir.AluOpType.not_equal, op1=MULT,
            )
            nc.gpsimd.tensor_tensor(
                bias_t[:], bias_t[:], maskC[:].to_broadcast([P, N, H]), op=ADD
            )

            at_ps = [ps_at.tile([P, 4, P], F32, tag="at", name=f"at{qt}_{i}") for i in range(2)]

            for h in range(H):
                tt_ = h // 8
                s_ = h % 2
                j_ = (h % 8) // 2
                fo, ro = h // 4, D * (h % 4)

                sc = ps_sc.tile([P, N], F32, tag="sc")
                kwargs = {}
                if ro == 96:
                    kwargs["tile_position"] = (ro, 0)
                nc.tensor.matmul(
                    sc[:],
                    qT_sb[ro : ro + D, fo, P * qt : P * (qt + 1)],
                    kT_sb[ro : ro + D, fo, :],
                    start=True,
                    stop=True,
                    **kwargs,
                )
                sp = spool.tile([P, N], F32, tag="sp")
                nc.vector.tensor_tensor(sp[:], sc[:], bias_t[:, :, h], op=ADD)

                for c in range(4):
                    nc.tensor.transpose(
                        sc[:, P * c : P * (c + 1)], sp[:, P * c : P * (c + 1)], ident[:]
                    )
                pT = ppool.tile([P, 4, P], F32, tag="pT")
                nc.scalar.activation(
                    pT[:], sc[:].rearrange("p (c q) -> p c q", c=4),
                    AF.Exp, bias=negc[:], scale=1.0,
                )

                for kc in range(4):
                    nc.tensor.matmul(
                        at_ps[tt_][64 * s_ : 64 * s_ + 2 * D, j_, :],
                        v_aug[:, kc, h, :],
                        pT[:, kc, :],
                        start=(kc == 0),
                        stop=(kc == 3),
                        tile_position=(0, 64 * s_),
                    )

            # ---- replicated rowsums -> reciprocal, then normalized attn^T
            rc = [rspool.tile([2 * D, 4, P], F32, tag=f"rc{i}", name=f"rc{qt}_{i}") for i in range(2)]
            for tt_ in range(2):
                for s_ in range(2):
                    nc.vector.reciprocal(
                        rc[tt_][D * s_ : D * (s_ + 1), :, :],
                        at_ps[tt_][64 * s_ + D : 64 * s_ + 2 * D, :, :],
                    )

            attnT_g = [atsb.tile([P, P], F32, tag=f"attnT{g}", name=f"attnT{qt}_{g}")
                       for g in range(4)]
            for h in range(H):
                tt_ = h // 8
                s_ = h % 2
                j_ = (h % 8) // 2
                g, ro = h // 4, D * (h % 4)
                nc.vector.tensor_tensor(
                    attnT_g[g][ro : ro + D, :],
                    at_ps[tt_][64 * s_ : 64 * s_ + D, j_, :],
                    rc[tt_][D * s_ : D * (s_ + 1), j_, :],
                    op=MULT,
                )

            # ---- output projection
            ps_o = ps_mi.tile([P, N], F32, tag="mi")
            for g in range(4):
                nc.tensor.matmul(
                    ps_o[:],
                    attnT_g[g][:],
                    w_sb["woT"][g][:],
                    start=(g == 0),
                    stop=False,
                )
            nc.tensor.matmul(ps_o[:], ones_r[:], bo_row[:], start=False, stop=True)
            o_sb = opool.tile([P, N], F32, tag="o")
            nc.scalar.copy(o_sb[:], ps_o[:])
            nc.sync.dma_start(t["out"][b, P * qt : P * (qt + 1), :], o_sb[:])


_PROG = None


def _get_prog(reps=1):
    global _PROG
    if reps != 1:
        return _build_prog(reps)
    if _PROG is None:
        _PROG = _build_prog(1)
    return _PROG


def _build_prog(reps):
        nc = bacc.Bacc("TRN2", target_bir_lowering=False, debug=False,
                       num_devices=NCORES)
        t = {
            "nfeat": nc.dram_tensor("nfeat", [BLOC, N, F], F32, kind="ExternalInput").ap(),
            "attn_bias": nc.dram_tensor("attn_bias", [BLOC, N, N, H], F32, kind="ExternalInput").ap(),
            "attn_mask": nc.dram_tensor("attn_mask", [BLOC, N, N], I32, kind="ExternalInput").ap(),
            "wqT": nc.dram_tensor("wqT", [F, F], F32, kind="ExternalInput").ap(),
            "wkT": nc.dram_tensor("wkT", [F, F], F32, kind="ExternalInput").ap(),
            "wvT": nc.dram_tensor("wvT", [F, F], F32, kind="ExternalInput").ap(),
            "woT": nc.dram_tensor("woT", [F, F], F32, kind="ExternalInput").ap(),
            "bqs": nc.dram_tensor("bqs", [F], F32, kind="ExternalInput").ap(),
            "bk": nc.dram_tensor("bk", [F], F32, kind="ExternalInput").ap(),
            "bv": nc.dram_tensor("bv", [F], F32, kind="ExternalInput").ap(),
            "bo": nc.dram_tensor("bo", [F], F32, kind="ExternalInput").ap(),
            "out": nc.dram_tensor("out", [BLOC, N, F], F32, kind="ExternalOutput").ap(),
        }
        with tile.TileContext(nc) as tc, ExitStack() as ctx:
            _emit(nc, tc, ctx, t, reps=reps)
        nc.compile()
        return nc


def kernel(nfeat, attn_bias, attn_mask, Wq, bq, Wk, bk, Wv, bv, Wo, bo):
    nc = _get_prog()
    nfeat = np.ascontiguousarray(np.asarray(nfeat, dtype=np.float32))
    attn_bias = np.ascontiguousarray(np.asarray(attn_bias, dtype=np.float32))
    attn_mask = np.ascontiguousarray(np.asarray(attn_mask, dtype=np.int32))
    shared = {
        "wqT": np.ascontiguousarray(np.asarray(Wq, dtype=np.float32).T),
        "wkT": np.ascontiguousarray(np.asarray(Wk, dtype=np.float32).T),
        "wvT": np.ascontiguousarray(np.asarray(Wv, dtype=np.float32).T),
        "woT": np.ascontiguousarray(np.asarray(Wo, dtype=np.float32).T),
        "bqs": np.asarray(bq, dtype=np.float32) * SQRT_D,
        "bk": np.asarray(bk, dtype=np.float32),
        "bv": np.asarray(bv, dtype=np.float32),
        "bo": np.asarray(bo, dtype=np.float32),
    }
    in_maps = []
    for c in range(NCORES):
        m = dict(shared)
        m["nfeat"] = nfeat[BLOC * c : BLOC * (c + 1)]
        m["attn_bias"] = attn_bias[BLOC * c : BLOC * (c + 1)]
        m["attn_mask"] = attn_mask[BLOC * c : BLOC * (c + 1)]
        in_maps.append(m)

    kernel.last_in_maps = in_maps
    trace = bool(int(os.environ.get("KERNEL_TRACE", "0")))
    res = run_bass_kernel_spmd(
        nc, in_maps, core_ids=list(range(NCORES)), trace=trace
    )
    if trace:
        kernel.last_exec_time_ns = res.exec_time_ns
        kernel.last_profile = res.profile_json
    out = np.concatenate([r["out"] for r in res.results], axis=0)
    return out.astype(np.float32)


kernel.last_exec_time_ns = None
kernel.last_profile = None
kernel.last_in_maps = None



# revision 29
# speedup vs baseline: 3.6709x; 3.6709x over previous
"""BiasedMHA Trainium2 kernel.

Full inputs -> shard batch over 8 NeuronCores -> Bass/Tile kernel -> gather.

Reference semantics (B=16, N=512, F=512, H=16, D=32):
  q = (x @ Wq.T + bq) * sqrt(D); k = x @ Wk.T + bk; v = x @ Wv.T + bv
  s[b,q,k,h] = sum_d q.k + bias[b,q,k,h];  s = -inf where mask[b,q,k]!=0
  p = softmax_k(s);  out = (p @ v reshaped) @ Wo.T + bo

Per-core design notes (cost-model driven):
 - All large matmuls use float32r moving operands (1 cyc/row when moving
   free size >= 256, vs 4 cyc/row for plain fp32) or 16-bit dtypes.
 - Scores stay q-major for the (q,k,h)-contiguous bias add, are written to
   SBUF as fp16 (precision) by the DVE bias-add, PE-transposed to k-major
   (fp16 identity as the moving operand -> 1 cyc/row), and exp'ed by ACT
   into bf16 probs (bf16 for exponent range: p ~ e^-30..e^-150).
 - Mask folding is split across engines to balance load: heads 0..11 get a
   POOL tensor_tensor fold of maskC into the bias tile; heads 12..15 get
   bias AND mask accumulated into the score PSUM by PE (identity-stationary
   matmuls, f32r moving) and a plain ACT fp16 copy instead of the DVE add.
 - V is kept per-head augmented with a bf16 ones column so P@V also emits
   the softmax denominator; normalization is folded into the at->SBUF copies.
 - softmax uses a fixed exp shift (exp(s - C)); masked adds use -60000 so
   every intermediate stays finite in fp16.
 - PSUM budget (8 banks): pair pool (2-bank tiles) x2, spt x2 (1 bank),
   at x2 (1 bank).
"""

import os
import numpy as np
from contextlib import ExitStack

import concourse.bass as bass
import concourse.mybir as mybir
import concourse.tile as tile
from concourse import bacc
from concourse.bass_utils import run_bass_kernel_spmd
from concourse.masks import make_identity

F32 = mybir.dt.float32
F32R = mybir.dt.float32r
F16 = mybir.dt.float16
BF16 = mybir.dt.bfloat16
I32 = mybir.dt.int32
ADD = mybir.AluOpType.add
MULT = mybir.AluOpType.mult
AF = mybir.ActivationFunctionType

B, N, F, H = 16, 512, 512, 16
D = F // H            # 32
NCORES = 8
BLOC = B // NCORES    # 2
P = 128
QT = N // P           # 4 q tiles
SQRT_D = float(np.sqrt(D))
C_EXP = 90.0          # fixed softmax shift; scores+bias-C in [-150, -25]
NEG = -60000.0        # mask add; keeps all fp16 intermediates finite
NPOOLH = 12           # heads 0..NPOOLH-1: POOL mask fold into bias
NPEH = 4              # last NPEH heads: PE accumulates bias+mask, ACT copy
# heads in [NPOOLH, H-NPEH): DVE adds raw bias, PE accumulates mask only


def _emit(nc, tc, ctx, t, reps=1):
    consts = ctx.enter_context(tc.tile_pool(name="consts", bufs=1))
    wpool = ctx.enter_context(tc.tile_pool(name="weights", bufs=1))
    xpool = ctx.enter_context(tc.tile_pool(name="x", bufs=5))
    bpool = ctx.enter_context(tc.tile_pool(name="perbatch", bufs=1))
    biaspool = ctx.enter_context(tc.tile_pool(name="bias", bufs=2))
    maskpool = ctx.enter_context(tc.tile_pool(name="mask", bufs=2))
    mcpool = ctx.enter_context(tc.tile_pool(name="maskC", bufs=2))
    sppool = ctx.enter_context(tc.tile_pool(name="sp", bufs=4))
    ppool = ctx.enter_context(tc.tile_pool(name="pT", bufs=3))
    atsb = ctx.enter_context(tc.tile_pool(name="attnT", bufs=2))
    opool = ctx.enter_context(tc.tile_pool(name="o", bufs=2))
    rspool = ctx.enter_context(tc.tile_pool(name="rs", bufs=2))

    # PSUM (8 banks): shared single-bank ring x4 (projections + scores),
    # transposed-pair tiles x2, attention accumulators x2
    ps1 = ctx.enter_context(tc.tile_pool(name="ps1", bufs=4, space="PSUM"))
    ps_spt = ctx.enter_context(tc.tile_pool(name="ps_spt", bufs=2, space="PSUM"))
    ps_at = ctx.enter_context(tc.tile_pool(name="ps_at", bufs=1, space="PSUM"))

    ident_f = consts.tile([P, P], F32)
    make_identity(nc, ident_f[:])
    ident_r = consts.tile([P, P], F32R)
    nc.vector.tensor_copy(ident_r[:], ident_f[:])
    ident_h = consts.tile([P, P], F16)
    nc.vector.tensor_copy(ident_h[:], ident_f[:])
    negc = consts.tile([P, 1], F32)
    nc.vector.memset(negc[:], -C_EXP)
    ones_col = consts.tile([1, P], F32)
    nc.vector.memset(ones_col[:], 1.0)
    ones_r = consts.tile([1, P], F32R)
    nc.vector.tensor_copy(ones_r[:], ones_col[:])

    # per-partition bias vectors for Q/K projection epilogues
    bqs_sb = consts.tile([P, 4], F32)
    nc.sync.dma_start(bqs_sb[:], t["bqs"].rearrange("(a p) -> p a", p=P))
    bk_sb = consts.tile([P, 4], F32)
    nc.sync.dma_start(bk_sb[:], t["bk"].rearrange("(a p) -> p a", p=P))
    bv_row = consts.tile([1, F], F32R)
    nc.sync.dma_start(bv_row[:], t["bv"].rearrange("(a f) -> a f", a=1))
    bo_row = consts.tile([1, F], F32R)
    nc.sync.dma_start(bo_row[:], t["bo"].rearrange("(a f) -> a f", a=1))

    # startup DMA order: x + q/k/v weights first (unblock projections), then
    # mask + first bias tile (quartered), then wo
    x_prefetch = []
    for nb in range(4):
        xt_ = xpool.tile([P, F], F32R, tag="x", name=f"xpre{nb}")
        nc.sync.dma_start(xt_[:], t["nfeat"][0, P * nb : P * (nb + 1), :])
        x_prefetch.append(xt_)
    mask_pre = maskpool.tile([P, N], I32, tag="mask", name="maskpre")
    nc.sync.dma_start(mask_pre[:], t["attn_mask"][0, 0:P, :])
    bias_pre = biaspool.tile([P, N, H], F32R, tag="bias", name="biaspre")
    for kq in range(2):
        nc.sync.dma_start(
            bias_pre[:, P * kq : P * (kq + 1), :],
            t["attn_bias"][0, 0:P, P * kq : P * (kq + 1), :],
        )
    w_sb = {}
    for name in ("wqT", "wkT"):
        w_sb[name] = []
        for ki in range(4):
            wt = wpool.tile([P, F], F32R, tag=f"{name}{ki}")
            nc.sync.dma_start(wt[:], t[name][P * ki : P * (ki + 1), :])
            w_sb[name].append(wt)
    for kq in range(2, 4):
        nc.sync.dma_start(
            bias_pre[:, P * kq : P * (kq + 1), :],
            t["attn_bias"][0, 0:P, P * kq : P * (kq + 1), :],
        )
    for name in ("wvT", "woT"):
        w_sb[name] = []
        for ki in range(4):
            wt = wpool.tile([P, F], F32R, tag=f"{name}{ki}")
            nc.sync.dma_start(wt[:], t[name][P * ki : P * (ki + 1), :])
            w_sb[name].append(wt)

    oproj_pend = None
    for rep in range(reps):
      for b in range(BLOC):
        # ---- X load + transpose to (f_in, n); paired PSUM + paired copies
        if rep == 0 and b == 0:
            x_tiles = x_prefetch
        else:
            x_tiles = []
            for nb in range(4):
                xt_ = xpool.tile([P, F], F32R, tag="x")
                nc.sync.dma_start(xt_[:], t["nfeat"][b, P * nb : P * (nb + 1), :])
                x_tiles.append(xt_)
        xT_sb = bpool.tile([P, 4, N], F32R, tag="xT")
        for fb in range(4):
            ps = ps1.tile([P, N], F32R, tag="ps")
            for nb in range(4):
                nc.tensor.transpose(
                    ps[:, P * nb : P * (nb + 1)],
                    x_tiles[nb][:, P * fb : P * (fb + 1)],
                    ident_r[:],
                )
            nc.scalar.copy(xT_sb[:, fb, :], ps[:])

        # ---- Q/K projections -> (f_out, n)
        qT_sb = bpool.tile([P, 4, N], F32R, tag="qT", bufs=2)
        kT_sb = bpool.tile([P, 4, N], F32R, tag="kT")
        for wname, dest, scale, bvec in (
            ("wqT", qT_sb, SQRT_D, bqs_sb),
            ("wkT", kT_sb, 1.0, bk_sb),
        ):
            for fo in range(4):
                ps = ps1.tile([P, N], F32, tag="ps")
                for ki in range(4):
                    nc.tensor.matmul(
                        ps[:],
                        w_sb[wname][ki][:, P * fo : P * (fo + 1)],
                        xT_sb[:, ki, :],
                        start=(ki == 0),
                        stop=(ki == 3),
                    )
                nc.scalar.activation(
                    dest[:, fo, :], ps[:], AF.Identity,
                    bias=bvec[:, fo : fo + 1], scale=scale,
                )

        # ---- V -> natural (n, f), bf16, augmented with ones columns
        v_aug = bpool.tile([P, 4, H, 2 * D], BF16, tag="vaug")
        for nb in range(4):
            ps = ps1.tile([P, N], F32, tag="ps")
            for ki in range(4):
                nc.tensor.matmul(
                    ps[:],
                    xT_sb[:, ki, P * nb : P * (nb + 1)],
                    w_sb["wvT"][ki][:],
                    start=(ki == 0),
                    stop=False,
                )
            nc.tensor.matmul(ps[:], ones_r[:], bv_row[:], start=False, stop=True)
            nc.scalar.copy(
                v_aug[:, nb, :, 0:D], ps[:].rearrange("p (h d) -> p h d", h=H)
            )
        nc.vector.memset(v_aug[:, :, :, D : 2 * D], 1.0)

        # ---- attention per q-tile, software-pipelined head-pair loop
        for qt in range(QT):
            if rep == 0 and b == 0 and qt == 0:
                # first q-tile: bias/mask were prefetched with the weights
                maskf = mcpool.tile([P, N], F32R, tag="maskf")
                nc.gpsimd.tensor_copy(maskf[:], mask_pre[:])
                maskC0 = mcpool.tile([P, N, 1], F32R, tag="maskC")
                nc.vector.tensor_scalar(
                    maskC0[:, :, 0], maskf[:], 0.0, NEG,
                    op0=mybir.AluOpType.not_equal, op1=MULT,
                )
                for kq in range(4):
                    nc.gpsimd.tensor_tensor(
                        bias_pre[:, P * kq : P * (kq + 1), 0:NPOOLH],
                        bias_pre[:, P * kq : P * (kq + 1), 0:NPOOLH],
                        maskC0[:, P * kq : P * (kq + 1), :].to_broadcast(
                            [P, P, NPOOLH]
                        ),
                        op=ADD,
                    )
                pend = (bias_pre, maskC0)

            bias_t, maskC = pend

            # prefetch + POOL-fold the NEXT q-tile while this one computes;
            # mask DMA first (small), bias quartered so the fold chases it
            nqt = qt + 1
            nb_, nq_ = (b, nqt) if nqt < QT else (b + 1, 0)
            wrap = False
            if nq_ == 0 and nb_ >= BLOC:
                nb_, wrap = 0, True
            if not (wrap and rep == reps - 1):
                mask_n = maskpool.tile([P, N], I32, tag="mask")
                nc.sync.dma_start(
                    mask_n[:], t["attn_mask"][nb_, P * nq_ : P * (nq_ + 1), :]
                )
                maskf_n = mcpool.tile([P, N], F32R, tag="maskf")
                nc.gpsimd.tensor_copy(maskf_n[:], mask_n[:])
                maskC_n = mcpool.tile([P, N, 1], F32R, tag="maskC")
                nc.vector.tensor_scalar(
                    maskC_n[:, :, 0], maskf_n[:], 0.0, NEG,
                    op0=mybir.AluOpType.not_equal, op1=MULT,
                )
                bias_n = biaspool.tile([P, N, H], F32R, tag="bias")
                for kq in range(4):
                    nc.sync.dma_start(
                        bias_n[:, P * kq : P * (kq + 1), :],
                        t["attn_bias"][
                            nb_, P * nq_ : P * (nq_ + 1), P * kq : P * (kq + 1), :
                        ],
                    )
                    nc.gpsimd.tensor_tensor(
                        bias_n[:, P * kq : P * (kq + 1), 0:NPOOLH],
                        bias_n[:, P * kq : P * (kq + 1), 0:NPOOLH],
                        maskC_n[:, P * kq : P * (kq + 1), :].to_broadcast(
                            [P, P, NPOOLH]
                        ),
                        op=ADD,
                    )
                pend = (bias_n, maskC_n)

            at_ps = [ps_at.tile([P, 4, P], F32, tag=f"at{i}", name=f"at{qt}_{i}")
                     for i in range(2)]

            rc = [rspool.tile([2 * D, 4, P], F32, tag=f"rc{i}", name=f"rc{qt}_{i}")
                  for i in range(2)]
            attnT_g = [atsb.tile([P, P], F32R, tag=f"attnT{g}", name=f"attnT{qt}_{g}")
                       for g in range(4)]

            def finish_tt(tt_):
                # reciprocal of replicated rowsums + normalized attn^T copies
                # for the 8 heads of accumulator tt_; frees at_ps[tt_]
                for s_ in range(2):
                    nc.vector.reciprocal(
                        rc[tt_][D * s_ : D * (s_ + 1), :, :],
                        at_ps[tt_][64 * s_ + D : 64 * s_ + 2 * D, :, :],
                    )
                for h in range(8 * tt_, 8 * tt_ + 8):
                    s_ = h % 2
                    j_ = (h % 8) // 2
                    g, ro = h // 4, D * (h % 4)
                    nc.vector.tensor_tensor(
                        attnT_g[g][ro : ro + D, :],
                        at_ps[tt_][64 * s_ : 64 * s_ + D, j_, :],
                        rc[tt_][D * s_ : D * (s_ + 1), j_, :],
                        op=MULT,
                    )

            # heads 12-15 first: they don't read the folded bias, giving the
            # POOL fold of this tile more slack and a faster q-tile start
            horder = list(range(NPOOLH, H)) + list(range(NPOOLH))
            sps, spts, pTs = {}, {}, {}
            ttdone, ttfin = set(), set()
            for step in range(H + 5):
                # stage 0: scores (+ PE bias/mask for late heads) -> sp
                if step < H:
                    h = horder[step]
                    fo, ro = h // 4, D * (h % 4)
                    pe_mask = h >= NPOOLH          # PE accumulates maskC
                    pe_bias = h >= H - NPEH        # PE also accumulates bias
                    sc = ps1.tile([P, N], F32, tag="ps")
                    kwargs = {}
                    if ro == 96:
                        kwargs["tile_position"] = (ro, 0)
                    nc.tensor.matmul(
                        sc[:],
                        qT_sb[ro : ro + D, fo, P * qt : P * (qt + 1)],
                        kT_sb[ro : ro + D, fo, :],
                        start=True,
                        stop=not pe_mask,
                        **kwargs,
                    )
                    if pe_bias:
                        nc.tensor.matmul(
                            sc[:], ident_r[:], bias_t[:, :, h],
                            start=False, stop=False,
                        )
                    if pe_mask:
                        nc.tensor.matmul(
                            sc[:], ident_r[:], maskC[:, :, 0],
                            start=False, stop=True,
                        )
                    sp = sppool.tile([P, N], F16, tag="sp")
                    if pe_bias:
                        nc.scalar.copy(sp[:], sc[:])
                    else:
                        nc.vector.tensor_tensor(
                            sp[:], sc[:], bias_t[:, :, h], op=ADD
                        )
                    sps[h] = sp

                    # cross-q-tile: emit the previous tile's output projection
                    # once this tile's pipeline is flowing
                    if step == 2 and oproj_pend is not None:
                        oproj_pend()
                        oproj_pend = None

                # stage 1: transpose position step-2 into its pair slot; exp
                # when the pair completes (pair parity == position parity)
                pos1 = step - 2
                if 0 <= pos1 < H:
                    h1 = horder[pos1]
                    sp = sps.pop(h1)
                    if pos1 % 2 == 0:
                        spts[pos1 // 2] = ps_spt.tile(
                            [P, 2, 4, P], F16, tag="spt", name=f"spt{qt}_{pos1 // 2}"
                        )
                    spt = spts[pos1 // 2]
                    for c in range(4):
                        nc.tensor.transpose(
                            spt[:, pos1 % 2, c, :], sp[:, P * c : P * (c + 1)],
                            ident_h[:],
                        )
                    if pos1 % 2 == 1:
                        pT = ppool.tile([P, 2, 4, P], BF16, tag="pT")
                        nc.scalar.activation(
                            pT[:], spts.pop(pos1 // 2)[:], AF.Exp,
                            bias=negc[:], scale=1.0,
                        )
                        pTs[pos1 // 2] = pT

                # stage 2: P@V for the pair finished two steps ago
                pos2 = step - 4
                if 0 <= pos2 < H and pos2 % 2 == 1:
                    pid = pos2 // 2
                    pT = pTs.pop(pid)
                    done_heads = []
                    for i in range(2):
                        h = horder[pos2 - 1 + i]
                        tt_ = h // 8
                        s_ = h % 2
                        j_ = (h % 8) // 2
                        done_heads.append(h)
                        for kc in range(4):
                            nc.tensor.matmul(
                                at_ps[tt_][64 * s_ : 64 * s_ + 2 * D, j_, :],
                                v_aug[:, kc, h, :],
                                pT[:, i, kc, :],
                                start=(kc == 0),
                                stop=(kc == 3),
                                tile_position=(0, 64 * s_),
                            )
                    ttdone.update(done_heads)
                    if all(h in ttdone for h in range(8)) and 0 not in ttfin:
                        ttfin.add(0)
                        finish_tt(0)  # heads 0-7 done: free at_ps[0] early
                    if all(h in ttdone for h in range(8, 16)) and 1 not in ttfin:
                        ttfin.add(1)
                        finish_tt(1)

            # ---- output projection: deferred into the next q-tile's stage
            # loop so PE doesn't block on the normalize chain at the boundary
            def make_oproj(attnT_g, b, qt):
                def emit():
                    ps_o = ps1.tile([P, N], F32, tag="ps", name=f"pso{b}_{qt}")
                    for g in range(4):
                        nc.tensor.matmul(
                            ps_o[:],
                            attnT_g[g][:],
                            w_sb["woT"][g][:],
                            start=(g == 0),
                            stop=False,
                        )
                    nc.tensor.matmul(
                        ps_o[:], ones_r[:], bo_row[:], start=False, stop=True
                    )
                    o_sb = opool.tile([P, N], F32, tag="o", name=f"osb{b}_{qt}")
                    nc.scalar.copy(o_sb[:], ps_o[:])
                    nc.sync.dma_start(
                        t["out"][b, P * qt : P * (qt + 1), :], o_sb[:]
                    )
                return emit

            if oproj_pend is not None:
                oproj_pend()
            oproj_pend = make_oproj(attnT_g, b, qt)

    if oproj_pend is not None:
        oproj_pend()


_PROG = None


def _get_prog(reps=1):
    global _PROG
    if reps != 1:
        return _build_prog(reps)
    if _PROG is None:
        _PROG = _build_prog(1)
    return _PROG


def _build_prog(reps):
        nc = bacc.Bacc("TRN2", target_bir_lowering=False, debug=False,
                       num_devices=NCORES)
        t = {
            "nfeat": nc.dram_tensor("nfeat", [BLOC, N, F], F32R, kind="ExternalInput").ap(),
            "attn_bias": nc.dram_tensor("attn_bias", [BLOC, N, N, H], F32R, kind="ExternalInput").ap(),
            "attn_mask": nc.dram_tensor("attn_mask", [BLOC, N, N], I32, kind="ExternalInput").ap(),
            "wqT": nc.dram_tensor("wqT", [F, F], F32R, kind="ExternalInput").ap(),
            "wkT": nc.dram_tensor("wkT", [F, F], F32R, kind="ExternalInput").ap(),
            "wvT": nc.dram_tensor("wvT", [F, F], F32R, kind="ExternalInput").ap(),
            "woT": nc.dram_tensor("woT", [F, F], F32R, kind="ExternalInput").ap(),
            "bqs": nc.dram_tensor("bqs", [F], F32, kind="ExternalInput").ap(),
            "bk": nc.dram_tensor("bk", [F], F32, kind="ExternalInput").ap(),
            "bv": nc.dram_tensor("bv", [F], F32R, kind="ExternalInput").ap(),
            "bo": nc.dram_tensor("bo", [F], F32R, kind="ExternalInput").ap(),
            "out": nc.dram_tensor("out", [BLOC, N, F], F32, kind="ExternalOutput").ap(),
        }
        with tile.TileContext(nc) as tc, ExitStack() as ctx:
            _emit(nc, tc, ctx, t, reps=reps)
        nc.compile()
        return nc


def kernel(nfeat, attn_bias, attn_mask, Wq, bq, Wk, bk, Wv, bv, Wo, bo):
    nc = _get_prog()
    nfeat = np.ascontiguousarray(np.asarray(nfeat, dtype=np.float32))
    attn_bias = np.ascontiguousarray(np.asarray(attn_bias, dtype=np.float32))
    attn_mask = np.ascontiguousarray(np.asarray(attn_mask, dtype=np.int32))
    shared = {
        "wqT": np.ascontiguousarray(np.asarray(Wq, dtype=np.float32).T),
        "wkT": np.ascontiguousarray(np.asarray(Wk, dtype=np.float32).T),
        "wvT": np.ascontiguousarray(np.asarray(Wv, dtype=np.float32).T),
        "woT": np.ascontiguousarray(np.asarray(Wo, dtype=np.float32).T),
        "bqs": np.asarray(bq, dtype=np.float32) * SQRT_D,
        "bk": np.asarray(bk, dtype=np.float32),
        "bv": np.asarray(bv, dtype=np.float32),
        "bo": np.asarray(bo, dtype=np.float32),
    }
    in_maps = []
    for c in range(NCORES):
        m = dict(shared)
        m["nfeat"] = nfeat[BLOC * c : BLOC * (c + 1)]
        m["attn_bias"] = attn_bias[BLOC * c : BLOC * (c + 1)]
        m["attn_mask"] = attn_mask[BLOC * c : BLOC * (c + 1)]
        in_maps.append(m)

    kernel.last_in_maps = in_maps
    trace = bool(int(os.environ.get("KERNEL_TRACE", "0")))
    res = run_bass_kernel_spmd(
        nc, in_maps, core_ids=list(range(NCORES)), trace=trace
    )
    if trace:
        kernel.last_exec_time_ns = res.exec_time_ns
        kernel.last_profile = res.profile_json
    out = np.concatenate([r["out"] for r in res.results], axis=0)
    return out.astype(np.float32)


kernel.last_exec_time_ns = None
kernel.last_profile = None
kernel.last_in_maps = None
